# revision 20
# baseline (speedup 1.0000x reference)
"""Trainium2 Bass kernel: AtomSelfInteraction GNN edge update.

out = silu(concat([h[idx_i], h[idx_j], m_ij], -1) @ W)

Strategy (8 NeuronCores, SPMD data-parallel over edges):
  - Each core owns E/8 = 25000 edges (padded to 196 x 128).
  - The row gathers h[idx_i], h[idx_j] are done on the HOST during input
    prep: per core we ship dense tensors hp_i / hp_j of shape
    [128, 2, e_pad] bf16 (feature-major, matmul-stationary layout), plus
    m_ij pre-transposed to [512, e_pad].  The device then runs a pure
    streaming pipeline: two fat hardware DMA queues (sync: m + hp_i,
    scalar: W + hp_j + out) feed the PE; there is no SWDGE gather, no
    descriptor-generation warmup, and no cross-core variance.
  - Device, per slab of 8 tiles: DMA m/hp_i/hp_j slabs; per 128-edge
    tile: 8 bf16 matmuls (K=1024 in 128-chunks) accumulate into a PSUM
    bank, SiLU on ScalarE (PSUM -> SBUF bf16), bf16 DMA out (host
    upcasts to f32).
  - First slab is phase-split (all m-chunks, then i, then j) and the PE
    is pre-warmed with junk matmuls so the p-state/HAM ramp overlaps the
    initial DMA wait.
"""

import numpy as np
import ml_dtypes

import concourse.tile as tile
from concourse import bacc
from concourse import mybir
from concourse.bass_utils import run_bass_kernel_spmd

P = 128
N_CORES = 8
E_TOTAL = 200000
EMB_ATOM = 256
EMB_EDGE = 512
IN_SIZE = 2 * EMB_ATOM + EMB_EDGE  # 1024

E_CORE = E_TOTAL // N_CORES        # 25000
TILES = (E_CORE + P - 1) // P      # 196
E_PAD = TILES * P                  # 25088

TILES_PER_SLAB = 8
WARMUP_MMS = 16

BF16 = mybir.dt.bfloat16
F32 = mybir.dt.float32

K_CHUNKS = IN_SIZE // P            # 8
M_CHUNKS = EMB_EDGE // P           # 4 (m_ij feature chunks, K chunks 4..7)
H_CHUNKS = EMB_ATOM // P           # 2 per h side


def build_nc(
    tiles=TILES,
    tiles_per_slab=TILES_PER_SLAB,
    act=mybir.ActivationFunctionType.Silu,
    out_dtype=BF16,
):
    e_pad = tiles * P
    nc = bacc.Bacc("TRN2", target_bir_lowering=False, debug=False)
    mt_d = nc.dram_tensor("m_t", [EMB_EDGE, e_pad], BF16, kind="ExternalInput").ap()
    hpi_d = nc.dram_tensor(
        "hp_i", [P, H_CHUNKS, e_pad], BF16, kind="ExternalInput"
    ).ap()
    hpj_d = nc.dram_tensor(
        "hp_j", [P, H_CHUNKS, e_pad], BF16, kind="ExternalInput"
    ).ap()
    w_d = nc.dram_tensor("w_bf", [IN_SIZE, EMB_EDGE], BF16, kind="ExternalInput").ap()
    out_d = nc.dram_tensor(
        "out", [e_pad, EMB_EDGE], out_dtype, kind="ExternalOutput"
    ).ap()

    # slab schedule: small phase-split first slab, steady slabs, small
    # last slab (short post-matmul activation/DMA tail)
    sizes = []
    rem = tiles
    first = min(4, rem)
    sizes.append(first)
    rem -= first
    last = 2 if rem > 2 else 0
    rem -= last
    while rem > 0:
        w = min(tiles_per_slab, rem)
        sizes.append(w)
        rem -= w
    if last:
        sizes.append(last)

    with tile.TileContext(nc) as tc:
        with (
            tc.tile_pool(name="const", bufs=1) as const_pool,
            tc.tile_pool(name="mt", bufs=3) as mt_pool,
            tc.tile_pool(name="hpi", bufs=3) as hpi_pool,
            tc.tile_pool(name="hpj", bufs=3) as hpj_pool,
            tc.tile_pool(name="warm", bufs=1, space="PSUM") as warm_pool,
            tc.tile_pool(name="acc", bufs=7, space="PSUM") as acc_pool,
            tc.tile_pool(name="outp", bufs=6) as out_pool,
        ):
            mt_r = mt_d.rearrange("(c p) e -> p c e", p=P)  # [128, 4, e_pad]
            w_r = w_d.rearrange("(k p) o -> p k o", p=P)

            # PE warmup: junk matmuls on a memset tile ramp the p-state /
            # HAM while the first real DMAs are still in flight.
            junk = const_pool.tile([P, EMB_EDGE], BF16, tag="junk")
            nc.gpsimd.memset(junk[:], 0.0)
            wacc = warm_pool.tile([32, EMB_EDGE], F32)
            for r in range(WARMUP_MMS):
                nc.tensor.matmul(
                    wacc[:],
                    lhsT=junk[:, :32],
                    rhs=junk[:],
                    start=(r == 0),
                    stop=(r == WARMUP_MMS - 1),
                )

            # First-feed critical path: the sync HW queue starts ~1.7us
            # before scalar, so it carries W's m-part and the first m
            # pieces; the warmup above is sized to end right as they land.
            # sync:   W-m, m0a, m0b, hp_i0, then per-slab m + hp_i
            # scalar: W-h, hp_j0, then per-slab hp_j + out tiles
            m0 = sizes[0] * P
            w_tile = const_pool.tile([P, K_CHUNKS, EMB_EDGE], BF16, tag="w")
            nc.sync.dma_start(w_tile[:, 2 * H_CHUNKS :], w_r[:, 2 * H_CHUNKS :])
            nc.scalar.dma_start(w_tile[:, : 2 * H_CHUNKS], w_r[:, : 2 * H_CHUNKS])
            mt0 = mt_pool.tile([P, M_CHUNKS, m0], BF16, tag="mt")
            h0 = m0 // 2
            nc.sync.dma_start(mt0[:, :, :h0], mt_r[:, :, :h0])
            nc.sync.dma_start(mt0[:, :, h0:], mt_r[:, :, h0:m0])
            hpj0 = hpj_pool.tile([P, H_CHUNKS, m0], BF16, tag="hpj")
            nc.scalar.dma_start(hpj0[:], hpj_d[:, :, :m0])
            hpi0 = hpi_pool.tile([P, H_CHUNKS, m0], BF16, tag="hpi")
            nc.sync.dma_start(hpi0[:], hpi_d[:, :, :m0])

            s0 = 0
            for si, nt in enumerate(sizes):
                e0 = s0 * P
                es = nt * P
                if si == 0:
                    mt_slab, hpi_t, hpj_t = mt0, hpi0, hpj0
                else:
                    mt_slab = mt_pool.tile([P, M_CHUNKS, es], BF16, tag="mt")
                    nc.sync.dma_start(mt_slab[:], mt_r[:, :, e0 : e0 + es])
                    hpi_t = hpi_pool.tile([P, H_CHUNKS, es], BF16, tag="hpi")
                    nc.sync.dma_start(hpi_t[:], hpi_d[:, :, e0 : e0 + es])
                    hpj_t = hpj_pool.tile([P, H_CHUNKS, es], BF16, tag="hpj")
                    nc.scalar.dma_start(hpj_t[:], hpj_d[:, :, e0 : e0 + es])

                if si == 0:
                    # phase-split: W h-part and hp_j arrive a few us after
                    # the m slab; m-chunks of all tiles run first
                    accs = []
                    for t in range(nt):
                        acc = acc_pool.tile([P, EMB_EDGE], F32)
                        accs.append(acc)
                        esl = slice(t * P, (t + 1) * P)
                        for c in range(M_CHUNKS):
                            nc.tensor.matmul(
                                acc[:], lhsT=mt_slab[:, c, esl],
                                rhs=w_tile[:, 2 * H_CHUNKS + c, :],
                                start=(c == 0), stop=False,
                            )
                    for t in range(nt):
                        esl = slice(t * P, (t + 1) * P)
                        for c in range(H_CHUNKS):
                            nc.tensor.matmul(
                                accs[t][:], lhsT=hpi_t[:, c, esl],
                                rhs=w_tile[:, c, :],
                                start=False, stop=False,
                            )
                    for t in range(nt):
                        esl = slice(t * P, (t + 1) * P)
                        for c in range(H_CHUNKS):
                            nc.tensor.matmul(
                                accs[t][:], lhsT=hpj_t[:, c, esl],
                                rhs=w_tile[:, H_CHUNKS + c, :],
                                start=False, stop=(c == H_CHUNKS - 1),
                            )
                        ot = out_pool.tile([P, EMB_EDGE], out_dtype)
                        nc.scalar.activation(ot[:], accs[t][:], act)
                        e_t = e0 + t * P
                        nc.scalar.dma_start(out_d[e_t : e_t + P, :], ot[:])
                else:
                    for t in range(nt):
                        acc = acc_pool.tile([P, EMB_EDGE], F32)
                        esl = slice(t * P, (t + 1) * P)
                        for c in range(M_CHUNKS):
                            nc.tensor.matmul(
                                acc[:], lhsT=mt_slab[:, c, esl],
                                rhs=w_tile[:, 2 * H_CHUNKS + c, :],
                                start=(c == 0), stop=False,
                            )
                        for c in range(H_CHUNKS):
                            nc.tensor.matmul(
                                acc[:], lhsT=hpi_t[:, c, esl],
                                rhs=w_tile[:, c, :],
                                start=False, stop=False,
                            )
                        for c in range(H_CHUNKS):
                            nc.tensor.matmul(
                                acc[:], lhsT=hpj_t[:, c, esl],
                                rhs=w_tile[:, H_CHUNKS + c, :],
                                start=False, stop=(c == H_CHUNKS - 1),
                            )
                        ot = out_pool.tile([P, EMB_EDGE], out_dtype)
                        nc.scalar.activation(ot[:], acc[:], act)
                        e_t = e0 + t * P
                        nc.scalar.dma_start(out_d[e_t : e_t + P, :], ot[:])
                s0 += nt
    nc.compile()
    return nc


def _feat_major(rows):
    """[e, 256] -> [128, 2, e] with hp[p, c, e] = rows[e, c*128 + p]."""
    e = rows.shape[0]
    return np.ascontiguousarray(
        rows.reshape(e, H_CHUNKS, P).transpose(2, 1, 0)
    )


def _ensure_ntff_hook():
    """Make trace=True work: register the ctypes NTFF profile hook when the
    image's antenv package lacks axon_hooks (boot degrades silently)."""
    import sys
    import types

    try:
        from antenv.axon_hooks import get_axon_ntff_profile_hook  # noqa: F401

        return
    except ImportError:
        pass
    import antenv
    from trn_agent_boot.trn_boot import _ntff_profile_via_ctypes

    hook = _ntff_profile_via_ctypes("/opt/axon/libaxon_pjrt.so")
    mod = types.ModuleType("antenv.axon_hooks")
    mod.get_axon_ntff_profile_hook = lambda: hook
    mod.set_axon_ntff_profile_hook = lambda h: None
    sys.modules["antenv.axon_hooks"] = mod
    antenv.axon_hooks = mod


_NC_CACHE = {}


def kernel(h, m_ij, idx_i, idx_j, W, trace=False):
    e_total = m_ij.shape[0]
    e_core = e_total // N_CORES
    tiles = (e_core + P - 1) // P
    e_pad = tiles * P
    if trace:
        _ensure_ntff_hook()

    h_bf = np.asarray(h).astype(ml_dtypes.bfloat16)
    w_bf = np.asarray(W).astype(ml_dtypes.bfloat16)
    m_bf = np.asarray(m_ij).astype(ml_dtypes.bfloat16)
    idx_i = np.asarray(idx_i)
    idx_j = np.asarray(idx_j)

    key = (tiles,)
    if key not in _NC_CACHE:
        _NC_CACHE[key] = build_nc(tiles=tiles)
    nc = _NC_CACHE[key]

    pad = e_pad - e_core
    in_maps = []
    for c in range(N_CORES):
        sl = slice(c * e_core, (c + 1) * e_core)
        m_pad = np.concatenate(
            [m_bf[sl], np.zeros((pad, EMB_EDGE), ml_dtypes.bfloat16)]
        ) if pad else m_bf[sl]
        ii = np.concatenate([idx_i[sl], np.zeros(pad, idx_i.dtype)]) if pad \
            else idx_i[sl]
        jj = np.concatenate([idx_j[sl], np.zeros(pad, idx_j.dtype)]) if pad \
            else idx_j[sl]
        in_maps.append({
            "m_t": np.ascontiguousarray(m_pad.T),
            "hp_i": _feat_major(h_bf[ii]),
            "hp_j": _feat_major(h_bf[jj]),
            "w_bf": w_bf,
        })

    res = run_bass_kernel_spmd(nc, in_maps, core_ids=list(range(N_CORES)), trace=trace)

    out = np.empty((e_total, EMB_EDGE), np.float32)
    for c in range(N_CORES):
        dev = res.results[c]["out"]
        out[c * e_core : (c + 1) * e_core] = dev[:e_core].astype(np.float32)
    if trace:
        kernel.last_result = res
    return out


# revision 23
# speedup vs baseline: 1.0050x; 1.0050x over previous
"""Trainium2 Bass kernel: AtomSelfInteraction GNN edge update.

out = silu(concat([h[idx_i], h[idx_j], m_ij], -1) @ W)

Strategy (8 NeuronCores, SPMD data-parallel over edges):
  - Each core owns E/8 = 25000 edges (padded to 196 x 128).
  - The row gathers h[idx_i], h[idx_j] are done on the HOST during input
    prep: per core we ship dense tensors hp_i / hp_j of shape
    [128, 2, e_pad] bf16 (feature-major, matmul-stationary layout), plus
    m_ij pre-transposed to [512, e_pad].  The device then runs a pure
    streaming pipeline: two fat hardware DMA queues (sync: m + hp_i,
    scalar: W + hp_j + out) feed the PE; there is no SWDGE gather, no
    descriptor-generation warmup, and no cross-core variance.
  - Device, per slab of 8 tiles: DMA m/hp_i/hp_j slabs; per 128-edge
    tile: 8 bf16 matmuls (K=1024 in 128-chunks) accumulate into a PSUM
    bank, SiLU on ScalarE (PSUM -> SBUF bf16), bf16 DMA out (host
    upcasts to f32).
  - First slab is phase-split (all m-chunks, then i, then j) and the PE
    is pre-warmed with junk matmuls so the p-state/HAM ramp overlaps the
    initial DMA wait.
"""

import numpy as np
import ml_dtypes

import concourse.tile as tile
from concourse import bacc
from concourse import mybir
from concourse.bass_utils import run_bass_kernel_spmd

P = 128
N_CORES = 8
E_TOTAL = 200000
EMB_ATOM = 256
EMB_EDGE = 512
IN_SIZE = 2 * EMB_ATOM + EMB_EDGE  # 1024

E_CORE = E_TOTAL // N_CORES        # 25000
TILES = (E_CORE + P - 1) // P      # 196
E_PAD = TILES * P                  # 25088

TILES_PER_SLAB = 8
WARMUP_MMS = 24

BF16 = mybir.dt.bfloat16
F32 = mybir.dt.float32

K_CHUNKS = IN_SIZE // P            # 8
M_CHUNKS = EMB_EDGE // P           # 4 (m_ij feature chunks, K chunks 4..7)
H_CHUNKS = EMB_ATOM // P           # 2 per h side


def build_nc(
    tiles=TILES,
    tiles_per_slab=TILES_PER_SLAB,
    act=mybir.ActivationFunctionType.Silu,
    out_dtype=BF16,
):
    e_pad = tiles * P
    nc = bacc.Bacc("TRN2", target_bir_lowering=False, debug=False)
    mt_d = nc.dram_tensor("m_t", [EMB_EDGE, e_pad], BF16, kind="ExternalInput").ap()
    hpi_d = nc.dram_tensor(
        "hp_i", [P, H_CHUNKS, e_pad], BF16, kind="ExternalInput"
    ).ap()
    hpj_d = nc.dram_tensor(
        "hp_j", [P, H_CHUNKS, e_pad], BF16, kind="ExternalInput"
    ).ap()
    w_d = nc.dram_tensor("w_bf", [IN_SIZE, EMB_EDGE], BF16, kind="ExternalInput").ap()
    out_d = nc.dram_tensor(
        "out", [e_pad, EMB_EDGE], out_dtype, kind="ExternalOutput"
    ).ap()

    # slab schedule: small phase-split first slab, then steady slabs
    sizes = []
    rem = tiles
    first = min(4, rem)
    sizes.append(first)
    rem -= first
    while rem > 0:
        w = min(tiles_per_slab, rem)
        sizes.append(w)
        rem -= w

    with tile.TileContext(nc) as tc:
        with (
            tc.tile_pool(name="const", bufs=1) as const_pool,
            tc.tile_pool(name="mt", bufs=3) as mt_pool,
            tc.tile_pool(name="hpi", bufs=3) as hpi_pool,
            tc.tile_pool(name="hpj", bufs=3) as hpj_pool,
            tc.tile_pool(name="warm", bufs=1, space="PSUM") as warm_pool,
            tc.tile_pool(name="acc", bufs=7, space="PSUM") as acc_pool,
            tc.tile_pool(name="outp", bufs=6) as out_pool,
        ):
            mt_r = mt_d.rearrange("(c p) e -> p c e", p=P)  # [128, 4, e_pad]
            w_r = w_d.rearrange("(k p) o -> p k o", p=P)

            # PE warmup: junk matmuls on a memset tile ramp the p-state /
            # HAM while the first real DMAs are still in flight.
            junk = const_pool.tile([P, EMB_EDGE], BF16, tag="junk")
            nc.gpsimd.memset(junk[:], 0.0)
            wacc = warm_pool.tile([32, EMB_EDGE], F32)
            for r in range(WARMUP_MMS):
                nc.tensor.matmul(
                    wacc[:],
                    lhsT=junk[:, :32],
                    rhs=junk[:],
                    start=(r == 0),
                    stop=(r == WARMUP_MMS - 1),
                )

            # sync HW queue: m slab 0, hp_i slab 0, then per-slab m + hp_i
            # scalar HW queue: W (m-part first), hp_j slabs, out tiles
            m0 = sizes[0] * P
            mt0 = mt_pool.tile([P, M_CHUNKS, m0], BF16, tag="mt")
            nc.sync.dma_start(mt0[:], mt_r[:, :, :m0])
            w_tile = const_pool.tile([P, K_CHUNKS, EMB_EDGE], BF16, tag="w")
            nc.scalar.dma_start(w_tile[:, 2 * H_CHUNKS :], w_r[:, 2 * H_CHUNKS :])
            nc.scalar.dma_start(w_tile[:, : 2 * H_CHUNKS], w_r[:, : 2 * H_CHUNKS])
            hpi0 = hpi_pool.tile([P, H_CHUNKS, m0], BF16, tag="hpi")
            nc.sync.dma_start(hpi0[:], hpi_d[:, :, :m0])
            hpj0 = hpj_pool.tile([P, H_CHUNKS, m0], BF16, tag="hpj")
            nc.scalar.dma_start(hpj0[:], hpj_d[:, :, :m0])

            s0 = 0
            for si, nt in enumerate(sizes):
                e0 = s0 * P
                es = nt * P
                if si == 0:
                    mt_slab, hpi_t, hpj_t = mt0, hpi0, hpj0
                else:
                    mt_slab = mt_pool.tile([P, M_CHUNKS, es], BF16, tag="mt")
                    nc.sync.dma_start(mt_slab[:], mt_r[:, :, e0 : e0 + es])
                    hpi_t = hpi_pool.tile([P, H_CHUNKS, es], BF16, tag="hpi")
                    nc.sync.dma_start(hpi_t[:], hpi_d[:, :, e0 : e0 + es])
                    hpj_t = hpj_pool.tile([P, H_CHUNKS, es], BF16, tag="hpj")
                    nc.scalar.dma_start(hpj_t[:], hpj_d[:, :, e0 : e0 + es])

                if si == 0:
                    # phase-split: W h-part and hp_j arrive a few us after
                    # the m slab; m-chunks of all tiles run first
                    accs = []
                    for t in range(nt):
                        acc = acc_pool.tile([P, EMB_EDGE], F32)
                        accs.append(acc)
                        esl = slice(t * P, (t + 1) * P)
                        for c in range(M_CHUNKS):
                            nc.tensor.matmul(
                                acc[:], lhsT=mt_slab[:, c, esl],
                                rhs=w_tile[:, 2 * H_CHUNKS + c, :],
                                start=(c == 0), stop=False,
                            )
                    for t in range(nt):
                        esl = slice(t * P, (t + 1) * P)
                        for c in range(H_CHUNKS):
                            nc.tensor.matmul(
                                accs[t][:], lhsT=hpi_t[:, c, esl],
                                rhs=w_tile[:, c, :],
                                start=False, stop=False,
                            )
                    for t in range(nt):
                        esl = slice(t * P, (t + 1) * P)
                        for c in range(H_CHUNKS):
                            nc.tensor.matmul(
                                accs[t][:], lhsT=hpj_t[:, c, esl],
                                rhs=w_tile[:, H_CHUNKS + c, :],
                                start=False, stop=(c == H_CHUNKS - 1),
                            )
                        ot = out_pool.tile([P, EMB_EDGE], out_dtype)
                        nc.scalar.activation(ot[:], accs[t][:], act)
                        e_t = e0 + t * P
                        nc.scalar.dma_start(out_d[e_t : e_t + P, :], ot[:])
                else:
                    for t in range(nt):
                        acc = acc_pool.tile([P, EMB_EDGE], F32)
                        esl = slice(t * P, (t + 1) * P)
                        for c in range(M_CHUNKS):
                            nc.tensor.matmul(
                                acc[:], lhsT=mt_slab[:, c, esl],
                                rhs=w_tile[:, 2 * H_CHUNKS + c, :],
                                start=(c == 0), stop=False,
                            )
                        for c in range(H_CHUNKS):
                            nc.tensor.matmul(
                                acc[:], lhsT=hpi_t[:, c, esl],
                                rhs=w_tile[:, c, :],
                                start=False, stop=False,
                            )
                        for c in range(H_CHUNKS):
                            nc.tensor.matmul(
                                acc[:], lhsT=hpj_t[:, c, esl],
                                rhs=w_tile[:, H_CHUNKS + c, :],
                                start=False, stop=(c == H_CHUNKS - 1),
                            )
                        ot = out_pool.tile([P, EMB_EDGE], out_dtype)
                        nc.scalar.activation(ot[:], acc[:], act)
                        e_t = e0 + t * P
                        nc.scalar.dma_start(out_d[e_t : e_t + P, :], ot[:])
                s0 += nt
    nc.compile()
    return nc


def _feat_major(rows):
    """[e, 256] -> [128, 2, e] with hp[p, c, e] = rows[e, c*128 + p]."""
    e = rows.shape[0]
    return np.ascontiguousarray(
        rows.reshape(e, H_CHUNKS, P).transpose(2, 1, 0)
    )


def _ensure_ntff_hook():
    """Make trace=True work: register the ctypes NTFF profile hook when the
    image's antenv package lacks axon_hooks (boot degrades silently)."""
    import sys
    import types

    try:
        from antenv.axon_hooks import get_axon_ntff_profile_hook  # noqa: F401

        return
    except ImportError:
        pass
    import antenv
    from trn_agent_boot.trn_boot import _ntff_profile_via_ctypes

    hook = _ntff_profile_via_ctypes("/opt/axon/libaxon_pjrt.so")
    mod = types.ModuleType("antenv.axon_hooks")
    mod.get_axon_ntff_profile_hook = lambda: hook
    mod.set_axon_ntff_profile_hook = lambda h: None
    sys.modules["antenv.axon_hooks"] = mod
    antenv.axon_hooks = mod


_NC_CACHE = {}


def kernel(h, m_ij, idx_i, idx_j, W, trace=False):
    e_total = m_ij.shape[0]
    e_core = e_total // N_CORES
    tiles = (e_core + P - 1) // P
    e_pad = tiles * P
    if trace:
        _ensure_ntff_hook()

    h_bf = np.asarray(h).astype(ml_dtypes.bfloat16)
    w_bf = np.asarray(W).astype(ml_dtypes.bfloat16)
    m_bf = np.asarray(m_ij).astype(ml_dtypes.bfloat16)
    idx_i = np.asarray(idx_i)
    idx_j = np.asarray(idx_j)

    key = (tiles,)
    if key not in _NC_CACHE:
        _NC_CACHE[key] = build_nc(tiles=tiles)
    nc = _NC_CACHE[key]

    pad = e_pad - e_core
    in_maps = []
    for c in range(N_CORES):
        sl = slice(c * e_core, (c + 1) * e_core)
        m_pad = np.concatenate(
            [m_bf[sl], np.zeros((pad, EMB_EDGE), ml_dtypes.bfloat16)]
        ) if pad else m_bf[sl]
        ii = np.concatenate([idx_i[sl], np.zeros(pad, idx_i.dtype)]) if pad \
            else idx_i[sl]
        jj = np.concatenate([idx_j[sl], np.zeros(pad, idx_j.dtype)]) if pad \
            else idx_j[sl]
        in_maps.append({
            "m_t": np.ascontiguousarray(m_pad.T),
            "hp_i": _feat_major(h_bf[ii]),
            "hp_j": _feat_major(h_bf[jj]),
            "w_bf": w_bf,
        })

    res = run_bass_kernel_spmd(nc, in_maps, core_ids=list(range(N_CORES)), trace=trace)

    out = np.empty((e_total, EMB_EDGE), np.float32)
    for c in range(N_CORES):
        dev = res.results[c]["out"]
        out[c * e_core : (c + 1) * e_core] = dev[:e_core].astype(np.float32)
    if trace:
        kernel.last_result = res
    return out
